# revision 2
# baseline (speedup 1.0000x reference)
"""Trainium2 Bass kernel v6 for CoExDispProcessor (topk_masking).

v6 vs v4/v5:
  - e stored as one [128, 4(dr), 960] fp16 tile per channel, single-buffered:
    the exp stream only depends on spg slab arrival (no consumer ring).
  - den matmul groups + reciprocals emitted EARLY (only need e), so PE/ACT
    drain them while topk runs.
  - fine muls run IN PLACE (e <- e*u) after the den matmuls have read e:
    no p tiles at all (saves 17.6MB of SBUF traffic and 33KB of SBUF).
  - coarse 1/(1+t) on DVE reciprocal (keeps the ACT Exp table hot; no
    table thrashing during the exp stream).
"""

import os
import sys
from contextlib import ExitStack

import numpy as np

if "/opt/trn_rl_repo" not in sys.path:
    sys.path.insert(0, "/opt/trn_rl_repo")

import concourse.bass as bass
import concourse.bacc as bacc
import concourse.tile as tile
from concourse import mybir
from concourse.bass_utils import run_bass_kernel_spmd

F32 = mybir.dt.float32
FP16 = mybir.dt.float16
I32 = mybir.dt.int32
OP = mybir.AluOpType
ACT = mybir.ActivationFunctionType

B, D, H, W = 8, 48, 128, 240
HF, WF = 4 * H, 4 * W
N_CORES = 8

SCALE = float(2 ** 27)
BIAS = 805306368.0  # 0.75*2^30; packed ints are positive-normal f32 patterns
COST_CHUNKS = ((0, 128), (128, 240))
PACK_CHUNKS = ((0, 61), (61, 122), (122, 240))
HALVES = [(0, 122), (122, 240)]
N_DVE_MUL = 6  # mul channels on DVE; rest on gpsimd


def _act_raw(nc, func, out_ap, in_ap, bias=0.0, scale=1.0):
    eng = nc.scalar
    return eng.add_instruction(
        mybir.InstActivation(
            name=nc.get_next_instruction_name(),
            func=func,
            ins=[
                eng.lower_ap(in_ap),
                mybir.ImmediateValue(dtype=F32, value=bias),
                mybir.ImmediateValue(dtype=F32, value=scale),
                mybir.ImmediateValue(dtype=F32, value=0.0),
            ],
            outs=[eng.lower_ap(out_ap)],
        )
    )


def build_kernel(ctx: ExitStack, tc: tile.TileContext, out_d, cost_d, spg_d, id_d):
    nc = tc.nc

    cost_hdw = cost_d.transpose([1, 0, 2])
    spg_v = spg_d.rearrange("c (R dr) w -> c R dr w", dr=4)
    out_v = out_d.rearrange("(R dr) w -> R dr w", dr=4)

    persist = ctx.enter_context(tc.tile_pool(name="persist", bufs=1))
    raw_pool = ctx.enter_context(tc.tile_pool(name="raw", bufs=12))
    fin_pool = ctx.enter_context(tc.tile_pool(name="fin", bufs=2))
    r4_pool = ctx.enter_context(tc.tile_pool(name="r4p", bufs=8))
    psum_den = ctx.enter_context(tc.psum_pool(name="psd", bufs=6))
    psum = ctx.enter_context(tc.psum_pool(name="ps", bufs=2))

    # ---------------- input DMAs (sync; cost first) ----------------
    c_full = persist.tile([128, D, W], F32)
    for wa, wb in COST_CHUNKS:
        nc.sync.dma_start(c_full[:, :, wa:wb], cost_hdw[:, :, wa:wb])
    ident = persist.tile([128, 128], FP16)
    nc.sync.dma_start(ident[:], id_d)

    raw_tiles = {}
    for dr in range(4):
        for c in range(9):
            raw = raw_pool.tile([128, WF], F32, tag="raw")
            nc.sync.dma_start(raw[:], spg_v[c, :, dr, :])
            raw_tiles[(dr, c)] = raw

    iota_i = persist.tile([128, D], I32)
    nc.gpsimd.iota(iota_i[:], pattern=[[-1, D]], base=47, channel_multiplier=0)

    # ---------------- tiles ----------------
    e_tiles = []
    for c in range(9):
        e = persist.tile([128, 4, WF], FP16, tag=f"e{c}")
        e_tiles.append(e)

    def exp_dr(dr):
        for c in range(9):
            nc.scalar.activation(e_tiles[c][:, dr, :], raw_tiles[(dr, c)][:], ACT.Exp)

    # den matmuls early (only need e); all reciprocals AFTER the exp stream
    # so the ACT Exp table loads exactly once and Recip once (no thrashing).
    r4_tiles = {}
    den_tiles = {}

    def den_dr(dr):
        for hf in range(2):
            fa, fb = hf * 480, (hf + 1) * 480
            den_ps = psum_den.tile([128, 480], F32, tag="den")
            for c in range(9):
                nc.tensor.matmul(den_ps[:], ident[:], e_tiles[c][:, dr, fa:fb],
                                 start=(c == 0), stop=(c == 8))
            den_tiles[(dr, hf)] = den_ps

    def recips_all():
        for dr in range(4):
            for hf in range(2):
                r4 = r4_pool.tile([128, 480], F32, tag="r4")
                _act_raw(nc, ACT.Reciprocal, r4[:], den_tiles[(dr, hf)][:],
                         scale=0.25)
                r4_tiles[(dr, hf)] = r4

    def pack_chunk(wa, wb):
        p32 = c_full[:].bitcast(I32)
        iota_b = iota_i[:].unsqueeze(2)
        nc.vector.tensor_scalar(out=p32[:, :, wa:wb], in0=c_full[:, :, wa:wb],
                                scalar1=SCALE, scalar2=BIAS, op0=OP.mult, op1=OP.add)
        nc.vector.tensor_scalar(out=p32[:, :, wa:wb], in0=p32[:, :, wa:wb],
                                scalar1=-64, scalar2=None, op0=OP.bitwise_and)
        nc.vector.tensor_tensor(
            p32[:, :, wa:wb], p32[:, :, wa:wb],
            iota_b.broadcast_to([128, D, wb - wa]), op=OP.bitwise_or)

    v8 = persist.tile([128, W, 8], F32)
    i1f = persist.tile([128, W], F32)
    i2f = persist.tile([128, W], F32)
    j32 = persist.tile([128, W], I32)
    df = persist.tile([128, W], F32)
    texp = persist.tile([128, W], F32)
    rden = persist.tile([128, W], F32)
    numc = persist.tile([128, W], F32)
    d4pad = persist.tile([128, W + 2], FP16)
    rv0 = persist.tile([128, W + 2], FP16)
    rv2 = persist.tile([128, W + 2], FP16)
    urep = []
    for s in range(3):
        u = persist.tile([128, 4 * (W + 2)], FP16, tag=f"urep{s}")
        urep.append(u)
    nc.vector.memset(d4pad[:], 0.0)
    nc.vector.memset(rv0[:], 0.0)
    nc.vector.memset(rv2[:], 0.0)

    def topk_half(h):
        a, b = HALVES[h]
        for w in range(a, b):
            nc.vector.max(out=v8[:, w], in_=c_full[:, :, w])

    def coarse_s1(h):
        """DVE index/delta extraction (runs right after this half's max8)."""
        a, b = HALVES[h]
        sl = slice(a, b)
        v8i = v8[:].bitcast(I32)
        nc.vector.tensor_scalar(out=j32[:, sl], in0=v8i[:, sl, 0], scalar1=63,
                                scalar2=None, op0=OP.bitwise_and)
        nc.vector.tensor_copy(i1f[:, sl], j32[:, sl])
        nc.vector.tensor_scalar(out=j32[:, sl], in0=v8i[:, sl, 1], scalar1=63,
                                scalar2=None, op0=OP.bitwise_and)
        nc.vector.tensor_copy(i2f[:, sl], j32[:, sl])
        nc.vector.tensor_tensor(df[:, sl], v8i[:, sl, 1], v8i[:, sl, 0], op=OP.subtract)

    def coarse_texp(h):
        a, b = HALVES[h]
        sl = slice(a, b)
        _act_raw(nc, ACT.Exp, texp[:, sl], df[:, sl], scale=1.0 / SCALE)

    def coarse_blend(h):
        a, b = HALVES[h]
        sl = slice(a, b)
        nc.vector.tensor_scalar_add(rden[:, sl], texp[:, sl], 1.0)
        nc.vector.reciprocal(rden[:, sl], rden[:, sl])
        nc.gpsimd.tensor_mul(numc[:, sl], texp[:, sl], i2f[:, sl])
        nc.gpsimd.tensor_add(numc[:, sl], numc[:, sl], i1f[:, sl])
        nc.gpsimd.tensor_mul(numc[:, sl], numc[:, sl], rden[:, sl])
        nc.gpsimd.tensor_scalar(out=d4pad[:, 1 + a:1 + b], in0=numc[:, sl],
                                scalar1=-1.0, scalar2=47.0, op0=OP.mult, op1=OP.add)
        nc.sync.dma_start(rv0[1:128, 1 + a:1 + b], d4pad[0:127, 1 + a:1 + b])
        nc.sync.dma_start(rv2[0:127, 1 + a:1 + b], d4pad[1:128, 1 + a:1 + b])

    def urep_half(h):
        xa, xb = (0, 122) if h == 0 else (120, 242)
        eng = nc.vector
        for s, rv in enumerate((rv0, d4pad, rv2)):
            eng.tensor_copy(
                urep[s][:, 4 * xa:4 * xb].rearrange("p (x dq) -> p x dq", dq=4),
                rv[:, xa:xb].unsqueeze(2).broadcast_to([128, xb - xa, 4]),
            )

    # ---------------- emission schedule ----------------
    exp_dr(0)
    exp_dr(1)
    den_dr(0)
    pack_chunk(*PACK_CHUNKS[0])
    pack_chunk(*PACK_CHUNKS[1])
    exp_dr(2)
    topk_half(0)
    coarse_s1(0)
    coarse_texp(0)
    pack_chunk(*PACK_CHUNKS[2])
    den_dr(1)
    exp_dr(3)
    coarse_blend(0)
    topk_half(1)
    coarse_s1(1)
    coarse_texp(1)
    den_dr(2)
    coarse_blend(1)
    urep_half(0)
    urep_half(1)
    den_dr(3)
    recips_all()

    # ---------------- fine: in-place muls + num matmuls + finals ----------
    for dr in range(4):
        for c in range(9):
            ci, cj = c // 3, c % 3
            usl = urep[ci][:, 4 * cj:4 * cj + WF]
            eng = nc.vector if c < N_DVE_MUL else nc.gpsimd
            eng.tensor_mul(e_tiles[c][:, dr, :], e_tiles[c][:, dr, :], usl)
        for hf in range(2):
            fa, fb = hf * 480, (hf + 1) * 480
            num_ps = psum.tile([128, 480], F32, tag="num")
            for c in range(9):
                nc.tensor.matmul(num_ps[:], ident[:], e_tiles[c][:, dr, fa:fb],
                                 start=(c == 0), stop=(c == 8))
            outt = fin_pool.tile([128, 480], F32, tag="outt")
            nc.vector.tensor_mul(outt[:], num_ps[:], r4_tiles[(dr, hf)][:])
            nc.sync.dma_start(out_v[:, dr, fa:fb], outt[:])


def build_program():
    nc = bacc.Bacc(
        "TRN2",
        target_bir_lowering=False,
        debug=False,
        enable_asserts=False,
        num_devices=N_CORES,
    )
    cost_d = nc.dram_tensor("cost", [D, H, W], F32, kind="ExternalInput").ap()
    spg_d = nc.dram_tensor("spg", [9, HF, WF], F32, kind="ExternalInput").ap()
    id_d = nc.dram_tensor("ident", [128, 128], FP16, kind="ExternalInput").ap()
    out_d = nc.dram_tensor("out", [HF, WF], F32, kind="ExternalOutput").ap()
    with tile.TileContext(nc) as tc:
        with ExitStack() as ctx:
            build_kernel(ctx, tc, out_d, cost_d, spg_d, id_d)
    nc.compile()
    return nc


def _install_ntff_hook():
    import types

    if "antenv.axon_hooks" in sys.modules:
        return True
    try:
        import antenv
        from trn_agent_boot.trn_boot import _ntff_profile_via_ctypes

        mod = types.ModuleType("antenv.axon_hooks")
        mod._hook = None

        def set_axon_ntff_profile_hook(hook):
            mod._hook = hook

        def get_axon_ntff_profile_hook():
            return mod._hook

        mod.set_axon_ntff_profile_hook = set_axon_ntff_profile_hook
        mod.get_axon_ntff_profile_hook = get_axon_ntff_profile_hook
        sys.modules["antenv.axon_hooks"] = mod
        antenv.axon_hooks = mod
        mod._hook = _ntff_profile_via_ctypes("/opt/axon/libaxon_pjrt.so")
        return True
    except Exception as e:
        print(f"NTFF hook install failed: {e}")
        return False


LAST_RESULTS = None


def kernel(cost: np.ndarray, spg: np.ndarray) -> np.ndarray:
    global LAST_RESULTS
    cost = np.ascontiguousarray(np.asarray(cost, dtype=np.float32))
    spg = np.ascontiguousarray(np.asarray(spg, dtype=np.float32))
    assert cost.shape == (B, 1, D, H, W) and spg.shape == (B, 9, HF, WF)

    nc = build_program()
    ident = np.eye(128, dtype=np.float16)
    in_maps = [
        {"cost": cost[b, 0], "spg": spg[b], "ident": ident} for b in range(B)
    ]
    trace = bool(int(os.environ.get("KERNEL_TRACE", "0")))
    if trace:
        trace = _install_ntff_hook()
    res = run_bass_kernel_spmd(
        nc, in_maps, core_ids=list(range(N_CORES)), trace=trace
    )
    LAST_RESULTS = res
    out = np.stack([res.results[b]["out"] for b in range(B)], axis=0)
    return out.astype(np.float32, copy=False)
